# revision 23
# baseline (speedup 1.0000x reference)
"""Sparse 2D-sliding-window + global-token attention block on 8 TRN2 NeuronCores.

Strategy: data-parallel over batch (B=8 -> one batch element per core, zero
collectives). Per core, for one [1032, 1024] sequence:

  - tokens reordered host-side: 1024 patches first (8 exact tiles of 128 =
    4 grid rows each), 8 special/CLS tokens last.  With that order, patch
    q-tile t only attends to patch k-tiles {t-1, t, t+1} plus the specials,
    and only 3 distinct 128x128 mask tiles exist.
  - QKV projection in bf16 (lhsT = X^T tiles, rhs = W^T), RMS-norm + RoPE in
    row layout (norm weights folded into host-precomputed cos/sin tables),
    then PE-transpose of q~/k~ into [d, m] layout for the score matmuls.
  - scores computed transposed (S^T = K~ Q~^T) so P^T = exp(S^T)*mask feeds
    the PV matmul directly; softmax uses no max-subtraction (RMS-normed rows
    have L2 norm exactly 8, so |s| <= 8 and exp(s/8) is safe); V carries an
    appended ones-column so denominators fall out of the PV matmul as row 64
    of O^T; the normalization fuses into the PSUM->SBUF copy-out.
  - out-projection consumes O^T directly as lhsT (no O transpose needed).
"""

import numpy as np
import ml_dtypes

B, N, DIM, HEADS, HD = 8, 1032, 1024, 16, 64
SPECIAL, GRID, WINDOW = 8, 32, 3
NP = 1024          # patch tokens
P = 128
NT = NP // P       # 8 patch tiles (4 grid rows each)
NC_ = DIM // P     # 8 contraction chunks
EPS = 1e-6
bf16 = ml_dtypes.bfloat16

_COMPILED = None


def _build():
    from contextlib import ExitStack
    import concourse.bass as bass
    import concourse.tile as tile
    from concourse import bacc, mybir
    from concourse.masks import make_identity

    dt = mybir.dt
    AF = mybir.ActivationFunctionType
    MUL = mybir.AluOpType.mult
    ADD = mybir.AluOpType.add

    nc = bacc.Bacc()

    xT = nc.declare_dram_parameter("xT", [P, NC_, N], dt.bfloat16, isOutput=False)
    wqkv = nc.declare_dram_parameter("wqkv", [P, NC_, 3 * DIM], dt.bfloat16, isOutput=False)
    wo = nc.declare_dram_parameter("wo", [P, NC_, DIM], dt.bfloat16, isOutput=False)
    # folded (norm-weight x cos/sin) tables, reordered to the m-layout, [128, 9, 64]
    cosq = nc.declare_dram_parameter("cosq", [P, NT + 1, HD], dt.bfloat16, isOutput=False)
    sinq = nc.declare_dram_parameter("sinq", [P, NT + 1, HD], dt.bfloat16, isOutput=False)
    cosk = nc.declare_dram_parameter("cosk", [P, NT + 1, HD], dt.bfloat16, isOutput=False)
    sink = nc.declare_dram_parameter("sink", [P, NT + 1, HD], dt.bfloat16, isOutput=False)
    msk = nc.declare_dram_parameter("msk", [P, 3 * P], dt.bfloat16, isOutput=False)
    out = nc.declare_dram_parameter("out", [N, DIM], dt.float32, isOutput=True)

    # m-tile geometry: tiles 0..7 are patches (128 rows), tile 8 is specials (8)
    def mslice(i):
        return slice(i * P, i * P + (P if i < NT else SPECIAL))

    def mp(i):
        return P if i < NT else SPECIAL

    with ExitStack() as ctx:
        ctx.enter_context(nc.allow_low_precision(reason="bf16 compute validated against f32 reference"))
        tc = ctx.enter_context(tile.TileContext(nc))
        persist = ctx.enter_context(tc.tile_pool(name="persist", bufs=1))
        temps = ctx.enter_context(tc.tile_pool(name="temps", bufs=3))
        ptp = ctx.enter_context(tc.tile_pool(name="ptp", bufs=10))
        oup = ctx.enter_context(tc.tile_pool(name="oup", bufs=20))
        ropep = ctx.enter_context(tc.tile_pool(name="ropep", bufs=4))
        temps2 = ctx.enter_context(tc.tile_pool(name="temps2", bufs=2))
        psum = ctx.enter_context(tc.tile_pool(name="psum", bufs=8, space="PSUM"))

        # ---- resident SBUF tensors -------------------------------------
        xT_sb = persist.tile([P, NC_, N], dt.bfloat16)
        nc.sync.dma_start(xT_sb[:], xT[:])
        wq_sb = persist.tile([P, NC_, 3 * DIM], dt.bfloat16)
        nc.sync.dma_start(wq_sb[:], wqkv[:])
        wo_sb = persist.tile([P, NC_, DIM], dt.bfloat16)
        nc.sync.dma_start(wo_sb[:], wo[:])
        tab = {}
        for nm, ap in (("cosq", cosq), ("sinq", sinq), ("cosk", cosk), ("sink", sink)):
            t = persist.tile([P, NT + 1, HD], dt.bfloat16, tag=f"tab_{nm}")
            nc.sync.dma_start(t[:], ap[:])
            tab[nm] = t
        msk_sb = persist.tile([P, 3 * P], dt.bfloat16)
        nc.sync.dma_start(msk_sb[:], msk[:])

        qT_sb = persist.tile([P, NC_, N], dt.bfloat16, tag="qT")
        kT_sb = persist.tile([P, NC_, N], dt.bfloat16, tag="kT")
        oT_sb = persist.tile([P, NC_, N], dt.bfloat16, tag="oT")
        # V with an interleaved ones column: [128, 9 m-tiles, 16 heads, 65]
        v_sb = persist.tile([P, NT + 1, HEADS, HD + 1], dt.bfloat16, tag="v")
        nc.vector.memset(v_sb[:, :, :, HD : HD + 1], 1.0)

        ident = persist.tile([P, P], dt.bfloat16, tag="ident")
        make_identity(nc, ident[:])
        onesT = persist.tile([97, HD], dt.bfloat16, tag="onesT")
        nc.vector.memset(onesT[:], 1.0)
        eps_sb = persist.tile([P, 1], dt.float32, tag="eps")
        nc.vector.memset(eps_sb[:], EPS)

        # ---- phase A: QKV projection + RMS norm + RoPE + transpose -----
        rope_pending = []

        def flush_transposes():
            for (ii, rope, dstT) in rope_pending:
                mm = mp(ii)
                mss = mslice(ii)
                for c2 in range(NC_):
                    ptr = psum.tile([P, 512], dt.bfloat16, tag="bank", name=f"tr{ii}_{c2}")
                    nc.tensor.transpose(
                        ptr[:P, :mm], rope[:mm, c2 * P : (c2 + 1) * P], ident[:mm, :mm]
                    )
                    nc.vector.tensor_copy(dstT[:, c2, mss], ptr[:P, :mm])
            rope_pending.clear()

        for i in range(NT + 1):
            m = mp(i)
            ms = mslice(i)
            ps_j = []
            for j in range(6):
                ps_j.append(psum.tile([P, 512], dt.float32, tag="bank", name=f"qkv_ps{j}"))
            for c in range(NC_):
                lhsT = xT_sb[:, c, ms]
                for j in range(6):
                    nc.tensor.matmul(
                        ps_j[j][:m, :],
                        lhsT,
                        wq_sb[:, c, j * 512 : (j + 1) * 512],
                        start=(c == 0),
                        stop=(c == NC_ - 1),
                    )
            flush_transposes()
            # V: copy into interleaved [head, 65] layout
            for j in (4, 5):
                nc.vector.tensor_copy(
                    v_sb[:m, i, (j - 4) * 8 : (j - 4) * 8 + 8, 0:HD],
                    ps_j[j][:m, :].rearrange("p (h d) -> p h d", h=8),
                )
            # Q (j=0,1) and K (j=2,3): norm + rope + transpose
            for which, (j0, cosn, sinn, dstT) in (
                ("q", (0, "cosq", "sinq", qT_sb)),
                ("k", (2, "cosk", "sink", kT_sb)),
            ):
                raw = temps2.tile([P, DIM], dt.bfloat16, tag="raw")
                for j in (j0, j0 + 1):
                    nc.scalar.copy(
                        raw[:m, (j - j0) * 512 : (j - j0 + 1) * 512], ps_j[j][:m, :]
                    )
                sq = temps2.tile([P, DIM], dt.bfloat16, tag="tsin")
                nc.scalar.activation(sq[:m], raw[:m], AF.Square)
                ssum = temps.tile([P, HEADS], dt.float32, tag="ssum")
                nc.vector.reduce_sum(
                    ssum[:m],
                    sq[:m].rearrange("p (h d) -> p h d", h=HEADS),
                    axis=mybir.AxisListType.X,
                )
                rstd = temps.tile([P, HEADS], dt.float32, tag="rstd")
                nc.scalar.activation(rstd[:m], ssum[:m], AF.Sqrt, bias=eps_sb[:m], scale=1.0 / HD)
                rst = temps.tile([P, HEADS], dt.bfloat16, tag="rst")
                nc.vector.reciprocal(rst[:m], rstd[:m])
                rv = raw[:m].rearrange("p (h two half) -> p h two half", h=HEADS, two=2)
                cosw = tab[cosn][:m, i, None, :].to_broadcast((m, HEADS, HD))
                sin0 = tab[sinn][:m, i, None, 0 : HD // 2].to_broadcast((m, HEADS, HD // 2))
                sin1 = tab[sinn][:m, i, None, HD // 2 : HD].to_broadcast((m, HEADS, HD // 2))
                tc_t = temps2.tile([P, DIM], dt.bfloat16, tag="tcos")
                nc.vector.tensor_tensor(
                    tc_t[:m].rearrange("p (h d) -> p h d", h=HEADS),
                    raw[:m].rearrange("p (h d) -> p h d", h=HEADS),
                    cosw,
                    op=MUL,
                )
                ts_t = temps2.tile([P, DIM], dt.bfloat16, tag="tsin")
                tsv = ts_t[:m].rearrange("p (h two half) -> p h two half", h=HEADS, two=2)
                nc.vector.tensor_tensor(tsv[:, :, 0, :], rv[:, :, 1, :], sin0, op=MUL)
                nc.vector.tensor_tensor(tsv[:, :, 1, :], rv[:, :, 0, :], sin1, op=MUL)
                nc.vector.tensor_tensor(tc_t[:m], tc_t[:m], ts_t[:m], op=ADD)
                rope = ropep.tile([P, DIM], dt.bfloat16, tag="rope")
                nc.vector.tensor_tensor(
                    rope[:m].rearrange("p (h d) -> p h d", h=HEADS),
                    tc_t[:m].rearrange("p (h d) -> p h d", h=HEADS),
                    rst[:m, :, None].to_broadcast((m, HEADS, HD)),
                    op=MUL,
                )
                rope_pending.append((i, rope, dstT))

        flush_transposes()

        # ---- phase B: banded attention, per head -----------------------
        norm_pending = [None]
        for h in range(HEADS):
            pb = 64 * (h % 2)
            ch = h // 2
            qTh = qT_sb[pb : pb + HD, ch, :]
            kTh = kT_sb[pb : pb + HD, ch, :]

            # S^T tiles; psum_st[t] free layout: [0:384) = s in {t-1,t,t+1},
            # [384:512) partitions 0:8 = specials-as-keys
            ps_st = [psum.tile([P, 512], dt.float32, tag="bank", name=f"st{_t}") for _t in range(NT)]
            ps_sp = psum.tile([P, 512], dt.float32, tag="bank")  # t=8 (special queries)
            for s in range(NT):
                lhsT = kTh[:, s * P : (s + 1) * P]
                for t in (s - 1, s, s + 1):
                    if 0 <= t < NT:
                        d = s - t + 1
                        nc.tensor.matmul(
                            ps_st[t][:P, d * P : (d + 1) * P],
                            lhsT,
                            qTh[:, t * P : (t + 1) * P],
                            start=True,
                            stop=True,
                        )
                nc.tensor.matmul(
                    ps_sp[:P, s * SPECIAL : (s + 1) * SPECIAL],
                    lhsT,
                    qTh[:, NP : NP + SPECIAL],
                    start=True,
                    stop=True,
                )
            lhsT_s = kTh[:, NP : NP + SPECIAL]
            ps_spk = [
                psum.tile([SPECIAL, 512], dt.float32, tag="bank", name=f"spk{_j}")
                for _j in range(2)
            ]
            for _j in range(2):
                nc.tensor.matmul(
                    ps_spk[_j][:SPECIAL, :], lhsT_s, qTh[:, _j * 512 : (_j + 1) * 512],
                    start=True, stop=True,
                )
            nc.tensor.matmul(
                ps_sp[:SPECIAL, NT * SPECIAL : NT * SPECIAL + SPECIAL],
                lhsT_s,
                qTh[:, NP : NP + SPECIAL],
                start=True,
                stop=True,
            )

            # exp (scale 1/sqrt(HD) folded) + mask -> P^T tiles in SBUF
            pts = []
            for t in range(NT):
                lo = 0 if t > 0 else P
                hi = 384 if t < NT - 1 else 256
                ptt = ptp.tile([P, 512], dt.bfloat16, tag="pt")
                nc.scalar.activation(
                    ptt[:P, lo:hi], ps_st[t][:P, lo:hi], AF.Exp, scale=0.125
                )
                nc.gpsimd.tensor_tensor(
                    ptt[:P, lo:hi], ptt[:P, lo:hi], msk_sb[:, lo:hi], op=MUL
                )
                pts.append(ptt)
            pt_spk = temps.tile([104, NP], dt.bfloat16, tag="ptspk")
            for _j in range(2):
                nc.scalar.activation(
                    pt_spk[:SPECIAL, _j * 512 : (_j + 1) * 512], ps_spk[_j][:SPECIAL, :],
                    AF.Exp, scale=0.125,
                )
            for _k in range(1, 4):
                nc.sync.dma_start(
                    pt_spk[32 * _k : 32 * _k + SPECIAL, :], pt_spk[:SPECIAL, :]
                )
            v_sp4 = temps.tile([104, HD + 1], dt.bfloat16, tag="vsp4")
            for _k in range(4):
                nc.scalar.copy(
                    v_sp4[32 * _k : 32 * _k + SPECIAL, :], v_sb[:SPECIAL, NT, h, :]
                )
            pt_sp = ptp.tile([P, 512], dt.bfloat16, tag="pt")
            nc.scalar.activation(
                pt_sp[:P, 0 : NT * SPECIAL], ps_sp[:P, 0 : NT * SPECIAL], AF.Exp, scale=0.125
            )
            nc.scalar.activation(
                pt_sp[:SPECIAL, NT * SPECIAL : (NT + 1) * SPECIAL],
                ps_sp[:SPECIAL, NT * SPECIAL : (NT + 1) * SPECIAL],
                AF.Exp,
                scale=0.125,
            )

            # PV: O^T(+denominator row) = [V | 1]^T @ P^T
            den0 = temps.tile([97, P], dt.float32, tag="den0")
            den1 = temps.tile([97, P], dt.float32, tag="den1")
            nc.vector.memset(den0[:], 1.0)
            nc.vector.memset(den1[:], 1.0)
            rec0 = temps.tile([97, P], dt.bfloat16, tag="rec0")
            rec1 = temps.tile([97, P], dt.bfloat16, tag="rec1")
            po_all = []
            for qd in ((0, 1, 2, 3), (4, 5, 6, 7)):
                t0 = qd[0]
                po_q = {
                    t: psum.tile([P, 512], dt.float32, tag="bank", name=f"po{t}")
                    for t in qd
                }
                for s in range(max(0, t0 - 1), min(NT, t0 + 5)):
                    for t in qd:
                        if abs(s - t) <= 1:
                            nc.tensor.matmul(
                                po_q[t][: HD + 1, :P],
                                v_sb[:, s, h, :],
                                pts[t][:P, (s - t + 1) * P : (s - t + 2) * P],
                                start=(s == max(0, t - 1)),
                                stop=False,
                                skip_group_check=True,
                            )
                for t in qd:
                    base = 32 * (t % 4)
                    nc.tensor.matmul(
                        po_q[t][: HD + 1, :P],
                        v_sp4[base : base + SPECIAL, :],
                        pt_spk[base : base + SPECIAL, t * P : (t + 1) * P],
                        start=False,
                        stop=True,
                        tile_position=(base, 0),
                        skip_group_check=True,
                    )
                for t in qd:
                    po = po_q[t]
                    dtile = den0 if t < 4 else den1
                    base = 32 * (t % 4)
                    nc.scalar.copy(dtile[base : base + 1, :P], po[HD : HD + 1, :P])
                    ou = oup.tile([HD, P], dt.bfloat16, tag="ou", name=f"ou{t}")
                    nc.scalar.copy(ou[:HD, :P], po[:HD, :P])
                    po_all.append(ou)
            # t = 8: special queries
            m = SPECIAL
            po = psum.tile([P, 512], dt.float32, tag="bank", name="po8")
            for k, s in enumerate(range(NT)):
                nc.tensor.matmul(
                    po[: HD + 1, :m], v_sb[:, s, h, :],
                    pt_sp[:P, s * SPECIAL : (s + 1) * SPECIAL],
                    start=(k == 0), stop=False,
                )
            nc.tensor.matmul(
                po[: HD + 1, :m], v_sb[:SPECIAL, NT, h, :],
                pt_sp[:SPECIAL, NT * SPECIAL : NT * SPECIAL + m],
                start=False, stop=True,
            )
            rec8 = temps.tile([1, SPECIAL], dt.bfloat16, tag="rec8")
            nc.vector.reciprocal(rec8[0:1, :m], po[HD : HD + 1, :m])
            ou = oup.tile([HD, P], dt.bfloat16, tag="ou", name="ou8")
            nc.scalar.copy(ou[:HD, :m], po[:HD, :m])
            po_all.append(ou)
            # batched exact reciprocals: 4 q-tiles per op at bases 0/32/64/96
            nc.vector.reciprocal(rec0[:], den0[:])
            nc.vector.reciprocal(rec1[:], den1[:])

            def make_norm(pb=pb, ch=ch, po_all=po_all, rec0=rec0, rec1=rec1, rec8=rec8):
                def emit():
                    for t in range(NT + 1):
                        m = mp(t)
                        po = po_all[t]
                        if t < NT:
                            rtile = rec0 if t < 4 else rec1
                            base = 32 * (t % 4)
                            rrow = rtile[base : base + 1, :m]
                        else:
                            base = 0
                            rrow = rec8[0:1, :m]
                        pb2 = psum.tile([P, 512], dt.float32, tag="bank", name=f"pb2_{t}")
                        nc.tensor.matmul(
                            pb2[:HD, :m], onesT[base : base + 1, :HD], rrow,
                            start=True, stop=True, tile_position=(base, 0),
                        )
                        nc.vector.tensor_tensor(
                            oT_sb[pb : pb + HD, ch, mslice(t)], po[:HD, :m], pb2[:HD, :m],
                            op=MUL,
                        )
                return emit

            if norm_pending[0] is not None:
                norm_pending[0]()
            norm_pending[0] = make_norm()

        if norm_pending[0] is not None:
            norm_pending[0]()

        # ---- phase C: out projection -----------------------------------
        for i in range(NT + 1):
            m = mp(i)
            row0 = SPECIAL + i * P if i < NT else 0
            for j in range(2):
                py = psum.tile([P, 512], dt.float32, tag="bank")
                for c in range(NC_):
                    nc.tensor.matmul(
                        py[:m, :],
                        oT_sb[:, c, mslice(i)],
                        wo_sb[:, c, j * 512 : (j + 1) * 512],
                        start=(c == 0),
                        stop=(c == NC_ - 1),
                    )
                y = temps.tile([P, 512], dt.float32, tag="y")
                nc.vector.tensor_copy(y[:m, :], py[:m, :])
                nc.sync.dma_start(out[row0 : row0 + m, j * 512 : (j + 1) * 512], y[:m, :])

    nc.compile()
    return nc


def _get_compiled():
    global _COMPILED
    if _COMPILED is None:
        _COMPILED = _build()
    return _COMPILED


def _tile_cm(a2d, nchunks):
    """[K, F] -> [128, K//128, F] with element [p, c, f] = a2d[c*128+p, f]."""
    K, F = a2d.shape
    return np.ascontiguousarray(
        a2d.reshape(nchunks, P, F).transpose(1, 0, 2)
    )


def _prep(freqs_cos, freqs_sin, qkv_w, out_w, norm_q_w, norm_k_w):
    perm = np.concatenate([np.arange(SPECIAL, N), np.arange(0, SPECIAL)])
    wqkv_t = _tile_cm(np.asarray(qkv_w, np.float32).T.astype(bf16), NC_)
    wo_t = _tile_cm(np.asarray(out_w, np.float32).T.astype(bf16), NC_)

    c_r = np.asarray(freqs_cos, np.float32)[perm]  # [1032, 64] in m-order
    s_r = np.asarray(freqs_sin, np.float32)[perm]
    h2 = HD // 2

    def fold(w):
        w = np.asarray(w, np.float32)
        cw = c_r * w[None, :]
        sw = np.empty_like(s_r)
        sw[:, :h2] = -s_r[:, :h2] * w[None, h2:]
        sw[:, h2:] = s_r[:, h2:] * w[None, :h2]
        return cw, sw

    cq, sq_ = fold(norm_q_w)
    ck, sk_ = fold(norm_k_w)

    def padtab(t):
        tp = np.zeros(((NT + 1) * P, HD), np.float32)
        tp[:N] = t
        return _tile_cm(tp.astype(bf16), NT + 1)

    # masks: tile[j(k-part), i(q-free)] for delta = s - t in (-1, 0, +1)
    jj, ii = np.meshgrid(np.arange(P), np.arange(P), indexing="ij")
    m3 = np.zeros((P, 3 * P), np.float32)
    for d in (-1, 0, 1):
        ok = (np.abs(4 * d + jj // GRID - ii // GRID) <= WINDOW) & (
            np.abs(jj % GRID - ii % GRID) <= WINDOW
        )
        m3[:, (d + 1) * P : (d + 2) * P] = ok
    return dict(
        wqkv=wqkv_t,
        wo=wo_t,
        cosq=padtab(cq),
        sinq=padtab(sq_),
        cosk=padtab(ck),
        sink=padtab(sk_),
        msk=m3.astype(bf16),
    )


def make_in_maps(hidden_states, freqs_cos, freqs_sin, qkv_w, out_w, norm_q_w, norm_k_w):
    shared = _prep(freqs_cos, freqs_sin, qkv_w, out_w, norm_q_w, norm_k_w)
    perm = np.concatenate([np.arange(SPECIAL, N), np.arange(0, SPECIAL)])
    hs = np.asarray(hidden_states, np.float32)
    in_maps = []
    for b in range(B):
        xb = hs[b][perm]                       # [1032, 1024] m-order
        xT = _tile_cm(np.ascontiguousarray(xb.T).astype(bf16), NC_)  # [128, 8, 1032]
        in_maps.append(dict(shared, xT=xT))
    return in_maps


def kernel(hidden_states, freqs_cos, freqs_sin, qkv_w, out_w, norm_q_w, norm_k_w):
    from concourse.bass_utils import run_bass_kernel_spmd

    nc = _get_compiled()
    in_maps = make_in_maps(
        hidden_states, freqs_cos, freqs_sin, qkv_w, out_w, norm_q_w, norm_k_w
    )
    res = run_bass_kernel_spmd(nc, in_maps, core_ids=list(range(B)))
    return np.stack([np.asarray(res.results[i]["out"], np.float32) for i in range(B)])


# revision 24
# speedup vs baseline: 1.0513x; 1.0513x over previous
"""Sparse 2D-sliding-window + global-token attention block on 8 TRN2 NeuronCores.

Strategy: data-parallel over batch (B=8 -> one batch element per core, zero
collectives). Per core, for one [1032, 1024] sequence:

  - tokens reordered host-side: 1024 patches first (8 exact tiles of 128 =
    4 grid rows each), 8 special/CLS tokens last.  With that order, patch
    q-tile t only attends to patch k-tiles {t-1, t, t+1} plus the specials,
    and only 3 distinct 128x128 mask tiles exist.
  - QKV projection in bf16 (lhsT = X^T tiles, rhs = W^T), RMS-norm + RoPE in
    row layout (norm weights folded into host-precomputed cos/sin tables),
    then PE-transpose of q~/k~ into [d, m] layout for the score matmuls.
  - scores computed transposed (S^T = K~ Q~^T) so P^T = exp(S^T)*mask feeds
    the PV matmul directly; softmax uses no max-subtraction (RMS-normed rows
    have L2 norm exactly 8, so |s| <= 8 and exp(s/8) is safe); V carries an
    appended ones-column so denominators fall out of the PV matmul as row 64
    of O^T; the normalization fuses into the PSUM->SBUF copy-out.
  - out-projection consumes O^T directly as lhsT (no O transpose needed).
"""

import numpy as np
import ml_dtypes

B, N, DIM, HEADS, HD = 8, 1032, 1024, 16, 64
SPECIAL, GRID, WINDOW = 8, 32, 3
NP = 1024          # patch tokens
P = 128
NT = NP // P       # 8 patch tiles (4 grid rows each)
NC_ = DIM // P     # 8 contraction chunks
EPS = 1e-6
bf16 = ml_dtypes.bfloat16

_COMPILED = None


def _build():
    from contextlib import ExitStack
    import concourse.bass as bass
    import concourse.tile as tile
    from concourse import bacc, mybir
    from concourse.masks import make_identity

    dt = mybir.dt
    AF = mybir.ActivationFunctionType
    MUL = mybir.AluOpType.mult
    ADD = mybir.AluOpType.add

    nc = bacc.Bacc()

    xT = nc.declare_dram_parameter("xT", [P, NC_, N], dt.bfloat16, isOutput=False)
    wqkv = nc.declare_dram_parameter("wqkv", [P, NC_, 3 * DIM], dt.bfloat16, isOutput=False)
    wo = nc.declare_dram_parameter("wo", [P, NC_, DIM], dt.bfloat16, isOutput=False)
    # folded (norm-weight x cos/sin) tables, reordered to the m-layout, [128, 9, 64]
    cosq = nc.declare_dram_parameter("cosq", [P, NT + 1, HD], dt.bfloat16, isOutput=False)
    sinq = nc.declare_dram_parameter("sinq", [P, NT + 1, HD], dt.bfloat16, isOutput=False)
    cosk = nc.declare_dram_parameter("cosk", [P, NT + 1, HD], dt.bfloat16, isOutput=False)
    sink = nc.declare_dram_parameter("sink", [P, NT + 1, HD], dt.bfloat16, isOutput=False)
    msk = nc.declare_dram_parameter("msk", [P, 3 * P], dt.bfloat16, isOutput=False)
    out = nc.declare_dram_parameter("out", [N, DIM], dt.float32, isOutput=True)

    # m-tile geometry: tiles 0..7 are patches (128 rows), tile 8 is specials (8)
    def mslice(i):
        return slice(i * P, i * P + (P if i < NT else SPECIAL))

    def mp(i):
        return P if i < NT else SPECIAL

    with ExitStack() as ctx:
        ctx.enter_context(nc.allow_low_precision(reason="bf16 compute validated against f32 reference"))
        tc = ctx.enter_context(tile.TileContext(nc))
        persist = ctx.enter_context(tc.tile_pool(name="persist", bufs=1))
        temps = ctx.enter_context(tc.tile_pool(name="temps", bufs=3))
        ptp = ctx.enter_context(tc.tile_pool(name="ptp", bufs=10))
        oup = ctx.enter_context(tc.tile_pool(name="oup", bufs=20))
        ropep = ctx.enter_context(tc.tile_pool(name="ropep", bufs=4))
        temps2 = ctx.enter_context(tc.tile_pool(name="temps2", bufs=2))
        psum = ctx.enter_context(tc.tile_pool(name="psum", bufs=8, space="PSUM"))

        # ---- resident SBUF tensors -------------------------------------
        xT_sb = persist.tile([P, NC_, N], dt.bfloat16)
        nc.sync.dma_start(xT_sb[:], xT[:])
        wq_sb = persist.tile([P, NC_, 3 * DIM], dt.bfloat16)
        nc.sync.dma_start(wq_sb[:], wqkv[:])
        wo_sb = persist.tile([P, NC_, DIM], dt.bfloat16)
        nc.sync.dma_start(wo_sb[:], wo[:])
        tab = {}
        for nm, ap in (("cosq", cosq), ("sinq", sinq), ("cosk", cosk), ("sink", sink)):
            t = persist.tile([P, NT + 1, HD], dt.bfloat16, tag=f"tab_{nm}")
            nc.sync.dma_start(t[:], ap[:])
            tab[nm] = t
        msk_sb = persist.tile([P, 3 * P], dt.bfloat16)
        nc.sync.dma_start(msk_sb[:], msk[:])

        qT_sb = persist.tile([P, NC_, N], dt.bfloat16, tag="qT")
        kT_sb = persist.tile([P, NC_, N], dt.bfloat16, tag="kT")
        oT_sb = persist.tile([P, NC_, N], dt.bfloat16, tag="oT")
        # V with an interleaved ones column: [128, 9 m-tiles, 16 heads, 65]
        v_sb = persist.tile([P, NT + 1, HEADS, HD + 1], dt.bfloat16, tag="v")
        nc.vector.memset(v_sb[:, :, :, HD : HD + 1], 1.0)

        ident = persist.tile([P, P], dt.bfloat16, tag="ident")
        make_identity(nc, ident[:])
        onesT = persist.tile([97, HD], dt.bfloat16, tag="onesT")
        nc.vector.memset(onesT[:], 1.0)
        eps_sb = persist.tile([P, 1], dt.float32, tag="eps")
        nc.vector.memset(eps_sb[:], EPS)

        # ---- phase A: QKV projection + RMS norm + RoPE + transpose -----
        rope_pending = []

        def flush_transposes():
            for (ii, rope, dstT) in rope_pending:
                mm = mp(ii)
                mss = mslice(ii)
                for c2 in range(NC_):
                    ptr = psum.tile([P, 512], dt.bfloat16, tag="bank", name=f"tr{ii}_{c2}")
                    nc.tensor.transpose(
                        ptr[:P, :mm], rope[:mm, c2 * P : (c2 + 1) * P], ident[:mm, :mm]
                    )
                    nc.vector.tensor_copy(dstT[:, c2, mss], ptr[:P, :mm])
            rope_pending.clear()

        for i in range(NT + 1):
            m = mp(i)
            ms = mslice(i)
            ps_j = []
            for j in range(6):
                ps_j.append(psum.tile([P, 512], dt.float32, tag="bank", name=f"qkv_ps{j}"))
            for c in range(NC_):
                lhsT = xT_sb[:, c, ms]
                for j in range(6):
                    nc.tensor.matmul(
                        ps_j[j][:m, :],
                        lhsT,
                        wq_sb[:, c, j * 512 : (j + 1) * 512],
                        start=(c == 0),
                        stop=(c == NC_ - 1),
                    )
            flush_transposes()
            # V: copy into interleaved [head, 65] layout
            for j in (4, 5):
                nc.vector.tensor_copy(
                    v_sb[:m, i, (j - 4) * 8 : (j - 4) * 8 + 8, 0:HD],
                    ps_j[j][:m, :].rearrange("p (h d) -> p h d", h=8),
                )
            # Q (j=0,1) and K (j=2,3): norm + rope + transpose
            for which, (j0, cosn, sinn, dstT) in (
                ("q", (0, "cosq", "sinq", qT_sb)),
                ("k", (2, "cosk", "sink", kT_sb)),
            ):
                raw = temps2.tile([P, DIM], dt.bfloat16, tag="raw")
                for j in (j0, j0 + 1):
                    nc.scalar.copy(
                        raw[:m, (j - j0) * 512 : (j - j0 + 1) * 512], ps_j[j][:m, :]
                    )
                sq = temps2.tile([P, DIM], dt.bfloat16, tag="tsin")
                nc.scalar.activation(sq[:m], raw[:m], AF.Square)
                ssum = temps.tile([P, HEADS], dt.float32, tag="ssum")
                nc.vector.reduce_sum(
                    ssum[:m],
                    sq[:m].rearrange("p (h d) -> p h d", h=HEADS),
                    axis=mybir.AxisListType.X,
                )
                rstd = temps.tile([P, HEADS], dt.float32, tag="rstd")
                nc.scalar.activation(rstd[:m], ssum[:m], AF.Sqrt, bias=eps_sb[:m], scale=1.0 / HD)
                rst = temps.tile([P, HEADS], dt.bfloat16, tag="rst")
                nc.vector.reciprocal(rst[:m], rstd[:m])
                rv = raw[:m].rearrange("p (h two half) -> p h two half", h=HEADS, two=2)
                cosw = tab[cosn][:m, i, None, :].to_broadcast((m, HEADS, HD))
                sin0 = tab[sinn][:m, i, None, 0 : HD // 2].to_broadcast((m, HEADS, HD // 2))
                sin1 = tab[sinn][:m, i, None, HD // 2 : HD].to_broadcast((m, HEADS, HD // 2))
                tc_t = temps2.tile([P, DIM], dt.bfloat16, tag="tcos")
                nc.vector.tensor_tensor(
                    tc_t[:m].rearrange("p (h d) -> p h d", h=HEADS),
                    raw[:m].rearrange("p (h d) -> p h d", h=HEADS),
                    cosw,
                    op=MUL,
                )
                ts_t = temps2.tile([P, DIM], dt.bfloat16, tag="tsin")
                tsv = ts_t[:m].rearrange("p (h two half) -> p h two half", h=HEADS, two=2)
                nc.vector.tensor_tensor(tsv[:, :, 0, :], rv[:, :, 1, :], sin0, op=MUL)
                nc.vector.tensor_tensor(tsv[:, :, 1, :], rv[:, :, 0, :], sin1, op=MUL)
                nc.vector.tensor_tensor(tc_t[:m], tc_t[:m], ts_t[:m], op=ADD)
                rope = ropep.tile([P, DIM], dt.bfloat16, tag="rope")
                nc.vector.tensor_tensor(
                    rope[:m].rearrange("p (h d) -> p h d", h=HEADS),
                    tc_t[:m].rearrange("p (h d) -> p h d", h=HEADS),
                    rst[:m, :, None].to_broadcast((m, HEADS, HD)),
                    op=MUL,
                )
                rope_pending.append((i, rope, dstT))

        flush_transposes()

        # ---- phase B: banded attention, per head -----------------------
        norm_pending = [None]
        for h in range(HEADS):
            pb = 64 * (h % 2)
            ch = h // 2
            qTh = qT_sb[pb : pb + HD, ch, :]
            kTh = kT_sb[pb : pb + HD, ch, :]

            # S^T computed per k-tile s against its contiguous 384-wide q-window
            # (one matmul per s), cols (t-s+1)*128 hold q-tile t
            ps_sp = psum.tile([P, 512], dt.float32, tag="bank")  # special queries
            pts = []
            for s in range(NT):
                lo = P if s == 0 else 0
                hi = 256 if s == NT - 1 else 384
                q0 = (s - 1) * P + lo
                lhsT = kTh[:, s * P : (s + 1) * P]
                st = psum.tile([P, 512], dt.float32, tag="bank", name=f"st{s}")
                nc.tensor.matmul(
                    st[:P, lo:hi], lhsT, qTh[:, q0 : q0 + hi - lo], start=True, stop=True
                )
                nc.tensor.matmul(
                    ps_sp[:P, s * SPECIAL : (s + 1) * SPECIAL],
                    lhsT,
                    qTh[:, NP : NP + SPECIAL],
                    start=True,
                    stop=True,
                )
                ptt = ptp.tile([P, 512], dt.bfloat16, tag="pt", name=f"pt{s}")
                nc.scalar.activation(ptt[:P, lo:hi], st[:P, lo:hi], AF.Exp, scale=0.125)
                nc.gpsimd.tensor_tensor(
                    ptt[:P, lo:hi], ptt[:P, lo:hi], msk_sb[:, lo:hi], op=MUL
                )
                pts.append(ptt)
            lhsT_s = kTh[:, NP : NP + SPECIAL]
            ps_spk = [
                psum.tile([SPECIAL, 512], dt.float32, tag="bank", name=f"spk{_j}")
                for _j in range(2)
            ]
            for _j in range(2):
                nc.tensor.matmul(
                    ps_spk[_j][:SPECIAL, :], lhsT_s, qTh[:, _j * 512 : (_j + 1) * 512],
                    start=True, stop=True,
                )
            nc.tensor.matmul(
                ps_sp[:SPECIAL, NT * SPECIAL : NT * SPECIAL + SPECIAL],
                lhsT_s,
                qTh[:, NP : NP + SPECIAL],
                start=True,
                stop=True,
            )
            pt_spk = temps.tile([SPECIAL, NP], dt.bfloat16, tag="ptspk")
            for _j in range(2):
                nc.scalar.activation(
                    pt_spk[:SPECIAL, _j * 512 : (_j + 1) * 512], ps_spk[_j][:SPECIAL, :],
                    AF.Exp, scale=0.125,
                )
            pt_sp = ptp.tile([P, 512], dt.bfloat16, tag="pt")
            nc.scalar.activation(
                pt_sp[:P, 0 : NT * SPECIAL], ps_sp[:P, 0 : NT * SPECIAL], AF.Exp, scale=0.125
            )
            nc.scalar.activation(
                pt_sp[:SPECIAL, NT * SPECIAL : (NT + 1) * SPECIAL],
                ps_sp[:SPECIAL, NT * SPECIAL : (NT + 1) * SPECIAL],
                AF.Exp,
                scale=0.125,
            )

            # PV: O^T(+denominator row) = [V | 1]^T @ P^T
            den0 = temps.tile([97, P], dt.float32, tag="den0")
            den1 = temps.tile([97, P], dt.float32, tag="den1")
            nc.vector.memset(den0[:], 1.0)
            nc.vector.memset(den1[:], 1.0)
            rec0 = temps.tile([97, P], dt.bfloat16, tag="rec0")
            rec1 = temps.tile([97, P], dt.bfloat16, tag="rec1")
            po_all = []
            for t in range(NT + 1):
                m = mp(t)
                ss = [s for s in (t - 1, t, t + 1) if 0 <= s < NT] if t < NT else list(range(NT))
                po = psum.tile([P, 512], dt.float32, tag="bank", name=f"po{t}")
                for k, s in enumerate(ss):
                    rhs = (
                        pts[s][:P, (t - s + 1) * P : (t - s + 2) * P]
                        if t < NT
                        else pt_sp[:P, s * SPECIAL : (s + 1) * SPECIAL]
                    )
                    nc.tensor.matmul(
                        po[: HD + 1, :m], v_sb[:, s, h, :], rhs,
                        start=(k == 0), stop=False,
                    )
                rhs_s = (
                    pt_spk[:SPECIAL, t * P : t * P + m]
                    if t < NT
                    else pt_sp[:SPECIAL, NT * SPECIAL : NT * SPECIAL + m]
                )
                nc.tensor.matmul(
                    po[: HD + 1, :m], v_sb[:SPECIAL, NT, h, :], rhs_s,
                    start=False, stop=True,
                )
                if t < NT:
                    dtile = den0 if t < 4 else den1
                    base = 32 * (t % 4)
                    nc.scalar.copy(dtile[base : base + 1, :m], po[HD : HD + 1, :m])
                else:
                    rec8 = temps.tile([1, SPECIAL], dt.bfloat16, tag="rec8")
                    nc.vector.reciprocal(rec8[0:1, :m], po[HD : HD + 1, :m])
                ou = oup.tile([HD, P], dt.bfloat16, tag="ou", name=f"ou{t}")
                nc.scalar.copy(ou[:HD, :m], po[:HD, :m])
                po_all.append(ou)
            # batched exact reciprocals: 4 q-tiles per op at bases 0/32/64/96
            nc.vector.reciprocal(rec0[:], den0[:])
            nc.vector.reciprocal(rec1[:], den1[:])

            def make_norm(pb=pb, ch=ch, po_all=po_all, rec0=rec0, rec1=rec1, rec8=rec8):
                def emit():
                    for t in range(NT + 1):
                        m = mp(t)
                        po = po_all[t]
                        if t < NT:
                            rtile = rec0 if t < 4 else rec1
                            base = 32 * (t % 4)
                            rrow = rtile[base : base + 1, :m]
                        else:
                            base = 0
                            rrow = rec8[0:1, :m]
                        pb2 = psum.tile([P, 512], dt.float32, tag="bank", name=f"pb2_{t}")
                        nc.tensor.matmul(
                            pb2[:HD, :m], onesT[base : base + 1, :HD], rrow,
                            start=True, stop=True, tile_position=(base, 0),
                        )
                        nc.vector.tensor_tensor(
                            oT_sb[pb : pb + HD, ch, mslice(t)], po[:HD, :m], pb2[:HD, :m],
                            op=MUL,
                        )
                return emit

            if norm_pending[0] is not None:
                norm_pending[0]()
            norm_pending[0] = make_norm()

        if norm_pending[0] is not None:
            norm_pending[0]()

        # ---- phase C: out projection -----------------------------------
        for i in range(NT + 1):
            m = mp(i)
            row0 = SPECIAL + i * P if i < NT else 0
            for j in range(2):
                py = psum.tile([P, 512], dt.float32, tag="bank")
                for c in range(NC_):
                    nc.tensor.matmul(
                        py[:m, :],
                        oT_sb[:, c, mslice(i)],
                        wo_sb[:, c, j * 512 : (j + 1) * 512],
                        start=(c == 0),
                        stop=(c == NC_ - 1),
                    )
                y = temps.tile([P, 512], dt.float32, tag="y")
                nc.vector.tensor_copy(y[:m, :], py[:m, :])
                nc.sync.dma_start(out[row0 : row0 + m, j * 512 : (j + 1) * 512], y[:m, :])

    nc.compile()
    return nc


def _get_compiled():
    global _COMPILED
    if _COMPILED is None:
        _COMPILED = _build()
    return _COMPILED


def _tile_cm(a2d, nchunks):
    """[K, F] -> [128, K//128, F] with element [p, c, f] = a2d[c*128+p, f]."""
    K, F = a2d.shape
    return np.ascontiguousarray(
        a2d.reshape(nchunks, P, F).transpose(1, 0, 2)
    )


def _prep(freqs_cos, freqs_sin, qkv_w, out_w, norm_q_w, norm_k_w):
    perm = np.concatenate([np.arange(SPECIAL, N), np.arange(0, SPECIAL)])
    wqkv_t = _tile_cm(np.asarray(qkv_w, np.float32).T.astype(bf16), NC_)
    wo_t = _tile_cm(np.asarray(out_w, np.float32).T.astype(bf16), NC_)

    c_r = np.asarray(freqs_cos, np.float32)[perm]  # [1032, 64] in m-order
    s_r = np.asarray(freqs_sin, np.float32)[perm]
    h2 = HD // 2

    def fold(w):
        w = np.asarray(w, np.float32)
        cw = c_r * w[None, :]
        sw = np.empty_like(s_r)
        sw[:, :h2] = -s_r[:, :h2] * w[None, h2:]
        sw[:, h2:] = s_r[:, h2:] * w[None, :h2]
        return cw, sw

    cq, sq_ = fold(norm_q_w)
    ck, sk_ = fold(norm_k_w)

    def padtab(t):
        tp = np.zeros(((NT + 1) * P, HD), np.float32)
        tp[:N] = t
        return _tile_cm(tp.astype(bf16), NT + 1)

    # masks: tile[j(k-part), i(q-free)] for delta = s - t in (-1, 0, +1)
    jj, ii = np.meshgrid(np.arange(P), np.arange(P), indexing="ij")
    m3 = np.zeros((P, 3 * P), np.float32)
    for d2 in (-1, 0, 1):
        ok = (np.abs(-4 * d2 + jj // GRID - ii // GRID) <= WINDOW) & (
            np.abs(jj % GRID - ii % GRID) <= WINDOW
        )
        m3[:, (d2 + 1) * P : (d2 + 2) * P] = ok
    return dict(
        wqkv=wqkv_t,
        wo=wo_t,
        cosq=padtab(cq),
        sinq=padtab(sq_),
        cosk=padtab(ck),
        sink=padtab(sk_),
        msk=m3.astype(bf16),
    )


def make_in_maps(hidden_states, freqs_cos, freqs_sin, qkv_w, out_w, norm_q_w, norm_k_w):
    shared = _prep(freqs_cos, freqs_sin, qkv_w, out_w, norm_q_w, norm_k_w)
    perm = np.concatenate([np.arange(SPECIAL, N), np.arange(0, SPECIAL)])
    hs = np.asarray(hidden_states, np.float32)
    in_maps = []
    for b in range(B):
        xb = hs[b][perm]                       # [1032, 1024] m-order
        xT = _tile_cm(np.ascontiguousarray(xb.T).astype(bf16), NC_)  # [128, 8, 1032]
        in_maps.append(dict(shared, xT=xT))
    return in_maps


def kernel(hidden_states, freqs_cos, freqs_sin, qkv_w, out_w, norm_q_w, norm_k_w):
    from concourse.bass_utils import run_bass_kernel_spmd

    nc = _get_compiled()
    in_maps = make_in_maps(
        hidden_states, freqs_cos, freqs_sin, qkv_w, out_w, norm_q_w, norm_k_w
    )
    res = run_bass_kernel_spmd(nc, in_maps, core_ids=list(range(B)))
    return np.stack([np.asarray(res.results[i]["out"], np.float32) for i in range(B)])


# revision 25
# speedup vs baseline: 1.0649x; 1.0130x over previous
"""Sparse 2D-sliding-window + global-token attention block on 8 TRN2 NeuronCores.

Strategy: data-parallel over batch (B=8 -> one batch element per core, zero
collectives). Per core, for one [1032, 1024] sequence:

  - tokens reordered host-side: 1024 patches first (8 exact tiles of 128 =
    4 grid rows each), 8 special/CLS tokens last.  With that order, patch
    q-tile t only attends to patch k-tiles {t-1, t, t+1} plus the specials,
    and only 3 distinct 128x128 mask tiles exist.
  - QKV projection in bf16 (lhsT = X^T tiles, rhs = W^T), RMS-norm + RoPE in
    row layout (norm weights folded into host-precomputed cos/sin tables),
    then PE-transpose of q~/k~ into [d, m] layout for the score matmuls.
  - scores computed transposed (S^T = K~ Q~^T) so P^T = exp(S^T)*mask feeds
    the PV matmul directly; softmax uses no max-subtraction (RMS-normed rows
    have L2 norm exactly 8, so |s| <= 8 and exp(s/8) is safe); V carries an
    appended ones-column so denominators fall out of the PV matmul as row 64
    of O^T; the normalization fuses into the PSUM->SBUF copy-out.
  - out-projection consumes O^T directly as lhsT (no O transpose needed).
"""

import numpy as np
import ml_dtypes

B, N, DIM, HEADS, HD = 8, 1032, 1024, 16, 64
SPECIAL, GRID, WINDOW = 8, 32, 3
NP = 1024          # patch tokens
P = 128
NT = NP // P       # 8 patch tiles (4 grid rows each)
NC_ = DIM // P     # 8 contraction chunks
EPS = 1e-6
bf16 = ml_dtypes.bfloat16

_COMPILED = None


def _build():
    from contextlib import ExitStack
    import concourse.bass as bass
    import concourse.tile as tile
    from concourse import bacc, mybir
    from concourse.masks import make_identity

    dt = mybir.dt
    AF = mybir.ActivationFunctionType
    MUL = mybir.AluOpType.mult
    ADD = mybir.AluOpType.add

    nc = bacc.Bacc()

    xT = nc.declare_dram_parameter("xT", [P, NC_, N], dt.bfloat16, isOutput=False)
    wqkv = nc.declare_dram_parameter("wqkv", [P, NC_, 3 * DIM], dt.bfloat16, isOutput=False)
    wo = nc.declare_dram_parameter("wo", [P, NC_, DIM], dt.bfloat16, isOutput=False)
    # folded (norm-weight x cos/sin) tables, reordered to the m-layout, [128, 9, 64]
    cosq = nc.declare_dram_parameter("cosq", [P, NT + 1, HD], dt.bfloat16, isOutput=False)
    sinq = nc.declare_dram_parameter("sinq", [P, NT + 1, HD], dt.bfloat16, isOutput=False)
    cosk = nc.declare_dram_parameter("cosk", [P, NT + 1, HD], dt.bfloat16, isOutput=False)
    sink = nc.declare_dram_parameter("sink", [P, NT + 1, HD], dt.bfloat16, isOutput=False)
    msk = nc.declare_dram_parameter("msk", [P, 3 * P], dt.bfloat16, isOutput=False)
    out = nc.declare_dram_parameter("out", [N, DIM], dt.float32, isOutput=True)

    # m-tile geometry: tiles 0..7 are patches (128 rows), tile 8 is specials (8)
    def mslice(i):
        return slice(i * P, i * P + (P if i < NT else SPECIAL))

    def mp(i):
        return P if i < NT else SPECIAL

    with ExitStack() as ctx:
        ctx.enter_context(nc.allow_low_precision(reason="bf16 compute validated against f32 reference"))
        tc = ctx.enter_context(tile.TileContext(nc))
        persist = ctx.enter_context(tc.tile_pool(name="persist", bufs=1))
        temps = ctx.enter_context(tc.tile_pool(name="temps", bufs=3))
        ptp = ctx.enter_context(tc.tile_pool(name="ptp", bufs=10))
        oup = ctx.enter_context(tc.tile_pool(name="oup", bufs=20))
        ropep = ctx.enter_context(tc.tile_pool(name="ropep", bufs=4))
        temps2 = ctx.enter_context(tc.tile_pool(name="temps2", bufs=2))
        psum = ctx.enter_context(tc.tile_pool(name="psum", bufs=8, space="PSUM"))

        # ---- resident SBUF tensors -------------------------------------
        xT_sb = persist.tile([P, NC_, N], dt.bfloat16)
        wq_sb = persist.tile([P, NC_, 3 * DIM], dt.bfloat16)
        wo_sb = persist.tile([P, NC_, DIM], dt.bfloat16)
        for c in range(NC_):
            nc.sync.dma_start(xT_sb[:, c, :], xT[:, c, :])
            nc.sync.dma_start(wq_sb[:, c, 0:1536], wqkv[:, c, 0:1536])
            nc.sync.dma_start(wq_sb[:, c, 1536:3072], wqkv[:, c, 1536:3072])
            nc.sync.dma_start(wo_sb[:, c, :], wo[:, c, :])
        tab = {}
        for nm, ap in (("cosq", cosq), ("sinq", sinq), ("cosk", cosk), ("sink", sink)):
            t = persist.tile([P, NT + 1, HD], dt.bfloat16, tag=f"tab_{nm}")
            nc.sync.dma_start(t[:], ap[:])
            tab[nm] = t
        msk_sb = persist.tile([P, 3 * P], dt.bfloat16)
        nc.sync.dma_start(msk_sb[:], msk[:])

        qT_sb = persist.tile([P, NC_, N], dt.bfloat16, tag="qT")
        kT_sb = persist.tile([P, NC_, N], dt.bfloat16, tag="kT")
        oT_sb = persist.tile([P, NC_, N], dt.bfloat16, tag="oT")
        # V with an interleaved ones column: [128, 9 m-tiles, 16 heads, 65]
        v_sb = persist.tile([P, NT + 1, HEADS, HD + 1], dt.bfloat16, tag="v")
        nc.vector.memset(v_sb[:, :, :, HD : HD + 1], 1.0)

        ident = persist.tile([P, P], dt.bfloat16, tag="ident")
        make_identity(nc, ident[:])
        onesT = persist.tile([97, HD], dt.bfloat16, tag="onesT")
        nc.vector.memset(onesT[:], 1.0)
        eps_sb = persist.tile([P, 1], dt.float32, tag="eps")
        nc.vector.memset(eps_sb[:], EPS)

        # ---- phase A: QKV projection + RMS norm + RoPE + transpose -----
        rope_pending = []

        def flush_transposes():
            for (ii, rope, dstT) in rope_pending:
                mm = mp(ii)
                mss = mslice(ii)
                for c2 in range(NC_):
                    ptr = psum.tile([P, 512], dt.bfloat16, tag="bank", name=f"tr{ii}_{c2}")
                    nc.tensor.transpose(
                        ptr[:P, :mm], rope[:mm, c2 * P : (c2 + 1) * P], ident[:mm, :mm]
                    )
                    nc.vector.tensor_copy(dstT[:, c2, mss], ptr[:P, :mm])
            rope_pending.clear()

        for i in range(NT + 1):
            m = mp(i)
            ms = mslice(i)
            ps_j = []
            for j in range(6):
                ps_j.append(psum.tile([P, 512], dt.float32, tag="bank", name=f"qkv_ps{j}"))
            for c in range(NC_):
                lhsT = xT_sb[:, c, ms]
                for j in range(6):
                    nc.tensor.matmul(
                        ps_j[j][:m, :],
                        lhsT,
                        wq_sb[:, c, j * 512 : (j + 1) * 512],
                        start=(c == 0),
                        stop=(c == NC_ - 1),
                    )
            flush_transposes()
            # V: copy into interleaved [head, 65] layout
            for j in (4, 5):
                nc.scalar.copy(
                    v_sb[:m, i, (j - 4) * 8 : (j - 4) * 8 + 8, 0:HD],
                    ps_j[j][:m, :].rearrange("p (h d) -> p h d", h=8),
                )
            # Q (j=0,1) and K (j=2,3): norm + rope + transpose
            for which, (j0, cosn, sinn, dstT) in (
                ("q", (0, "cosq", "sinq", qT_sb)),
                ("k", (2, "cosk", "sink", kT_sb)),
            ):
                raw = temps2.tile([P, DIM], dt.bfloat16, tag="raw")
                for j in (j0, j0 + 1):
                    nc.scalar.copy(
                        raw[:m, (j - j0) * 512 : (j - j0 + 1) * 512], ps_j[j][:m, :]
                    )
                sq = temps2.tile([P, DIM], dt.bfloat16, tag="tsin")
                nc.scalar.activation(sq[:m], raw[:m], AF.Square)
                ssum = temps.tile([P, HEADS], dt.float32, tag="ssum")
                nc.vector.reduce_sum(
                    ssum[:m],
                    sq[:m].rearrange("p (h d) -> p h d", h=HEADS),
                    axis=mybir.AxisListType.X,
                )
                rstd = temps.tile([P, HEADS], dt.float32, tag="rstd")
                nc.scalar.activation(rstd[:m], ssum[:m], AF.Sqrt, bias=eps_sb[:m], scale=1.0 / HD)
                rst = temps.tile([P, HEADS], dt.bfloat16, tag="rst")
                nc.vector.reciprocal(rst[:m], rstd[:m])
                rv = raw[:m].rearrange("p (h two half) -> p h two half", h=HEADS, two=2)
                cosw = tab[cosn][:m, i, None, :].to_broadcast((m, HEADS, HD))
                sin0 = tab[sinn][:m, i, None, 0 : HD // 2].to_broadcast((m, HEADS, HD // 2))
                sin1 = tab[sinn][:m, i, None, HD // 2 : HD].to_broadcast((m, HEADS, HD // 2))
                tc_t = temps2.tile([P, DIM], dt.bfloat16, tag="tcos")
                nc.vector.tensor_tensor(
                    tc_t[:m].rearrange("p (h d) -> p h d", h=HEADS),
                    raw[:m].rearrange("p (h d) -> p h d", h=HEADS),
                    cosw,
                    op=MUL,
                )
                ts_t = temps2.tile([P, DIM], dt.bfloat16, tag="tsin")
                tsv = ts_t[:m].rearrange("p (h two half) -> p h two half", h=HEADS, two=2)
                nc.vector.tensor_tensor(tsv[:, :, 0, :], rv[:, :, 1, :], sin0, op=MUL)
                nc.vector.tensor_tensor(tsv[:, :, 1, :], rv[:, :, 0, :], sin1, op=MUL)
                nc.vector.tensor_tensor(tc_t[:m], tc_t[:m], ts_t[:m], op=ADD)
                rope = ropep.tile([P, DIM], dt.bfloat16, tag="rope")
                nc.vector.tensor_tensor(
                    rope[:m].rearrange("p (h d) -> p h d", h=HEADS),
                    tc_t[:m].rearrange("p (h d) -> p h d", h=HEADS),
                    rst[:m, :, None].to_broadcast((m, HEADS, HD)),
                    op=MUL,
                )
                rope_pending.append((i, rope, dstT))

        flush_transposes()

        # ---- phase B: banded attention, per head -----------------------
        norm_pending = [None]
        for h in range(HEADS):
            pb = 64 * (h % 2)
            ch = h // 2
            qTh = qT_sb[pb : pb + HD, ch, :]
            kTh = kT_sb[pb : pb + HD, ch, :]

            # S^T computed per k-tile s against its contiguous 384-wide q-window
            # (one matmul per s), cols (t-s+1)*128 hold q-tile t
            ps_sp = psum.tile([P, 512], dt.float32, tag="bank")  # special queries
            pts = []
            for s in range(NT):
                lo = P if s == 0 else 0
                hi = 256 if s == NT - 1 else 384
                q0 = (s - 1) * P + lo
                lhsT = kTh[:, s * P : (s + 1) * P]
                st = psum.tile([P, 512], dt.float32, tag="bank", name=f"st{s}")
                nc.tensor.matmul(
                    st[:P, lo:hi], lhsT, qTh[:, q0 : q0 + hi - lo], start=True, stop=True
                )
                nc.tensor.matmul(
                    ps_sp[:P, s * SPECIAL : (s + 1) * SPECIAL],
                    lhsT,
                    qTh[:, NP : NP + SPECIAL],
                    start=True,
                    stop=True,
                )
                ptt = ptp.tile([P, 512], dt.bfloat16, tag="pt", name=f"pt{s}")
                nc.scalar.activation(ptt[:P, lo:hi], st[:P, lo:hi], AF.Exp, scale=0.125)
                nc.gpsimd.tensor_tensor(
                    ptt[:P, lo:hi], ptt[:P, lo:hi], msk_sb[:, lo:hi], op=MUL
                )
                pts.append(ptt)
            lhsT_s = kTh[:, NP : NP + SPECIAL]
            ps_spk = [
                psum.tile([SPECIAL, 512], dt.float32, tag="bank", name=f"spk{_j}")
                for _j in range(2)
            ]
            for _j in range(2):
                nc.tensor.matmul(
                    ps_spk[_j][:SPECIAL, :], lhsT_s, qTh[:, _j * 512 : (_j + 1) * 512],
                    start=True, stop=True,
                )
            nc.tensor.matmul(
                ps_sp[:SPECIAL, NT * SPECIAL : NT * SPECIAL + SPECIAL],
                lhsT_s,
                qTh[:, NP : NP + SPECIAL],
                start=True,
                stop=True,
            )
            pt_spk = temps.tile([SPECIAL, NP], dt.bfloat16, tag="ptspk")
            for _j in range(2):
                nc.scalar.activation(
                    pt_spk[:SPECIAL, _j * 512 : (_j + 1) * 512], ps_spk[_j][:SPECIAL, :],
                    AF.Exp, scale=0.125,
                )
            pt_sp = ptp.tile([P, 512], dt.bfloat16, tag="pt")
            nc.scalar.activation(
                pt_sp[:P, 0 : NT * SPECIAL], ps_sp[:P, 0 : NT * SPECIAL], AF.Exp, scale=0.125
            )
            nc.scalar.activation(
                pt_sp[:SPECIAL, NT * SPECIAL : (NT + 1) * SPECIAL],
                ps_sp[:SPECIAL, NT * SPECIAL : (NT + 1) * SPECIAL],
                AF.Exp,
                scale=0.125,
            )

            # PV: O^T(+denominator row) = [V | 1]^T @ P^T
            den0 = temps.tile([97, P], dt.float32, tag="den0")
            den1 = temps.tile([97, P], dt.float32, tag="den1")
            nc.vector.memset(den0[:], 1.0)
            nc.vector.memset(den1[:], 1.0)
            rec0 = temps.tile([97, P], dt.bfloat16, tag="rec0")
            rec1 = temps.tile([97, P], dt.bfloat16, tag="rec1")
            po_all = []
            for t in range(NT + 1):
                m = mp(t)
                ss = [s for s in (t - 1, t, t + 1) if 0 <= s < NT] if t < NT else list(range(NT))
                po = psum.tile([P, 512], dt.float32, tag="bank", name=f"po{t}")
                for k, s in enumerate(ss):
                    rhs = (
                        pts[s][:P, (t - s + 1) * P : (t - s + 2) * P]
                        if t < NT
                        else pt_sp[:P, s * SPECIAL : (s + 1) * SPECIAL]
                    )
                    nc.tensor.matmul(
                        po[: HD + 1, :m], v_sb[:, s, h, :], rhs,
                        start=(k == 0), stop=False,
                    )
                rhs_s = (
                    pt_spk[:SPECIAL, t * P : t * P + m]
                    if t < NT
                    else pt_sp[:SPECIAL, NT * SPECIAL : NT * SPECIAL + m]
                )
                nc.tensor.matmul(
                    po[: HD + 1, :m], v_sb[:SPECIAL, NT, h, :], rhs_s,
                    start=False, stop=True,
                )
                if t < NT:
                    dtile = den0 if t < 4 else den1
                    base = 32 * (t % 4)
                    nc.scalar.copy(dtile[base : base + 1, :m], po[HD : HD + 1, :m])
                else:
                    rec8 = temps.tile([1, SPECIAL], dt.bfloat16, tag="rec8")
                    nc.vector.reciprocal(rec8[0:1, :m], po[HD : HD + 1, :m])
                ou = oup.tile([HD, P], dt.bfloat16, tag="ou", name=f"ou{t}")
                nc.scalar.copy(ou[:HD, :m], po[:HD, :m])
                po_all.append(ou)
            # batched exact reciprocals: 4 q-tiles per op at bases 0/32/64/96
            nc.vector.reciprocal(rec0[:], den0[:])
            nc.vector.reciprocal(rec1[:], den1[:])

            def make_norm(pb=pb, ch=ch, po_all=po_all, rec0=rec0, rec1=rec1, rec8=rec8):
                def emit():
                    for t in range(NT + 1):
                        m = mp(t)
                        po = po_all[t]
                        if t < NT:
                            rtile = rec0 if t < 4 else rec1
                            base = 32 * (t % 4)
                            rrow = rtile[base : base + 1, :m]
                        else:
                            base = 0
                            rrow = rec8[0:1, :m]
                        pb2 = psum.tile([P, 512], dt.float32, tag="bank", name=f"pb2_{t}")
                        nc.tensor.matmul(
                            pb2[:HD, :m], onesT[base : base + 1, :HD], rrow,
                            start=True, stop=True, tile_position=(base, 0),
                        )
                        nc.vector.tensor_tensor(
                            oT_sb[pb : pb + HD, ch, mslice(t)], po[:HD, :m], pb2[:HD, :m],
                            op=MUL,
                        )
                return emit

            if norm_pending[0] is not None:
                norm_pending[0]()
            norm_pending[0] = make_norm()

        if norm_pending[0] is not None:
            norm_pending[0]()

        # ---- phase C: out projection -----------------------------------
        for i in range(NT + 1):
            m = mp(i)
            row0 = SPECIAL + i * P if i < NT else 0
            for j in range(2):
                py = psum.tile([P, 512], dt.float32, tag="bank")
                for c in range(NC_):
                    nc.tensor.matmul(
                        py[:m, :],
                        oT_sb[:, c, mslice(i)],
                        wo_sb[:, c, j * 512 : (j + 1) * 512],
                        start=(c == 0),
                        stop=(c == NC_ - 1),
                    )
                y = temps.tile([P, 512], dt.float32, tag="y")
                nc.scalar.copy(y[:m, :], py[:m, :])
                nc.sync.dma_start(out[row0 : row0 + m, j * 512 : (j + 1) * 512], y[:m, :])

    nc.compile()
    return nc


def _get_compiled():
    global _COMPILED
    if _COMPILED is None:
        _COMPILED = _build()
    return _COMPILED


def _tile_cm(a2d, nchunks):
    """[K, F] -> [128, K//128, F] with element [p, c, f] = a2d[c*128+p, f]."""
    K, F = a2d.shape
    return np.ascontiguousarray(
        a2d.reshape(nchunks, P, F).transpose(1, 0, 2)
    )


def _prep(freqs_cos, freqs_sin, qkv_w, out_w, norm_q_w, norm_k_w):
    perm = np.concatenate([np.arange(SPECIAL, N), np.arange(0, SPECIAL)])
    wqkv_t = _tile_cm(np.asarray(qkv_w, np.float32).T.astype(bf16), NC_)
    wo_t = _tile_cm(np.asarray(out_w, np.float32).T.astype(bf16), NC_)

    c_r = np.asarray(freqs_cos, np.float32)[perm]  # [1032, 64] in m-order
    s_r = np.asarray(freqs_sin, np.float32)[perm]
    h2 = HD // 2

    def fold(w):
        w = np.asarray(w, np.float32)
        cw = c_r * w[None, :]
        sw = np.empty_like(s_r)
        sw[:, :h2] = -s_r[:, :h2] * w[None, h2:]
        sw[:, h2:] = s_r[:, h2:] * w[None, :h2]
        return cw, sw

    cq, sq_ = fold(norm_q_w)
    ck, sk_ = fold(norm_k_w)

    def padtab(t):
        tp = np.zeros(((NT + 1) * P, HD), np.float32)
        tp[:N] = t
        return _tile_cm(tp.astype(bf16), NT + 1)

    # masks: tile[j(k-part), i(q-free)] for delta = s - t in (-1, 0, +1)
    jj, ii = np.meshgrid(np.arange(P), np.arange(P), indexing="ij")
    m3 = np.zeros((P, 3 * P), np.float32)
    for d2 in (-1, 0, 1):
        ok = (np.abs(-4 * d2 + jj // GRID - ii // GRID) <= WINDOW) & (
            np.abs(jj % GRID - ii % GRID) <= WINDOW
        )
        m3[:, (d2 + 1) * P : (d2 + 2) * P] = ok
    return dict(
        wqkv=wqkv_t,
        wo=wo_t,
        cosq=padtab(cq),
        sinq=padtab(sq_),
        cosk=padtab(ck),
        sink=padtab(sk_),
        msk=m3.astype(bf16),
    )


def make_in_maps(hidden_states, freqs_cos, freqs_sin, qkv_w, out_w, norm_q_w, norm_k_w):
    shared = _prep(freqs_cos, freqs_sin, qkv_w, out_w, norm_q_w, norm_k_w)
    perm = np.concatenate([np.arange(SPECIAL, N), np.arange(0, SPECIAL)])
    hs = np.asarray(hidden_states, np.float32)
    in_maps = []
    for b in range(B):
        xb = hs[b][perm]                       # [1032, 1024] m-order
        xT = _tile_cm(np.ascontiguousarray(xb.T).astype(bf16), NC_)  # [128, 8, 1032]
        in_maps.append(dict(shared, xT=xT))
    return in_maps


def kernel(hidden_states, freqs_cos, freqs_sin, qkv_w, out_w, norm_q_w, norm_k_w):
    from concourse.bass_utils import run_bass_kernel_spmd

    nc = _get_compiled()
    in_maps = make_in_maps(
        hidden_states, freqs_cos, freqs_sin, qkv_w, out_w, norm_q_w, norm_k_w
    )
    res = run_bass_kernel_spmd(nc, in_maps, core_ids=list(range(B)))
    return np.stack([np.asarray(res.results[i]["out"], np.float32) for i in range(B)])


# revision 28
# speedup vs baseline: 1.1204x; 1.0521x over previous
"""Sparse 2D-sliding-window + global-token attention block on 8 TRN2 NeuronCores.

Strategy: data-parallel over batch (B=8 -> one batch element per core, zero
collectives). Per core, for one [1032, 1024] sequence:

  - tokens reordered host-side: 1024 patches first (8 exact tiles of 128 =
    4 grid rows each), 8 special/CLS tokens last.  With that order, patch
    q-tile t only attends to patch k-tiles {t-1, t, t+1} plus the specials,
    and only 3 distinct 128x128 mask tiles exist.
  - QKV projection in bf16 (lhsT = X^T tiles, rhs = W^T), RMS-norm + RoPE in
    row layout (norm weights folded into host-precomputed cos/sin tables),
    then PE-transpose of q~/k~ into [d, m] layout for the score matmuls.
  - scores computed transposed (S^T = K~ Q~^T) so P^T = exp(S^T)*mask feeds
    the PV matmul directly; softmax uses no max-subtraction (RMS-normed rows
    have L2 norm exactly 8, so |s| <= 8 and exp(s/8) is safe); V carries an
    appended ones-column so denominators fall out of the PV matmul as row 64
    of O^T; the normalization fuses into the PSUM->SBUF copy-out.
  - out-projection consumes O^T directly as lhsT (no O transpose needed).
"""

import numpy as np
import ml_dtypes

B, N, DIM, HEADS, HD = 8, 1032, 1024, 16, 64
SPECIAL, GRID, WINDOW = 8, 32, 3
NP = 1024          # patch tokens
P = 128
NT = NP // P       # 8 patch tiles (4 grid rows each)
NC_ = DIM // P     # 8 contraction chunks
EPS = 1e-6
bf16 = ml_dtypes.bfloat16

_COMPILED = None


def _build():
    from contextlib import ExitStack
    import concourse.bass as bass
    import concourse.tile as tile
    from concourse import bacc, mybir
    from concourse.masks import make_identity

    dt = mybir.dt
    AF = mybir.ActivationFunctionType
    MUL = mybir.AluOpType.mult
    ADD = mybir.AluOpType.add

    nc = bacc.Bacc()

    xT = nc.declare_dram_parameter("xT", [P, NC_, N], dt.bfloat16, isOutput=False)
    wqkv = nc.declare_dram_parameter("wqkv", [P, NC_, 3 * DIM], dt.bfloat16, isOutput=False)
    wo = nc.declare_dram_parameter("wo", [P, NC_, DIM], dt.bfloat16, isOutput=False)
    # folded (norm-weight x cos/sin) tables, reordered to the m-layout, [128, 9, 64]
    cosq = nc.declare_dram_parameter("cosq", [P, NT + 1, HD], dt.bfloat16, isOutput=False)
    sinq = nc.declare_dram_parameter("sinq", [P, NT + 1, HD], dt.bfloat16, isOutput=False)
    cosk = nc.declare_dram_parameter("cosk", [P, NT + 1, HD], dt.bfloat16, isOutput=False)
    sink = nc.declare_dram_parameter("sink", [P, NT + 1, HD], dt.bfloat16, isOutput=False)
    msk = nc.declare_dram_parameter("msk", [P, 3 * P], dt.bfloat16, isOutput=False)
    out = nc.declare_dram_parameter("out", [N, DIM], dt.float32, isOutput=True)

    # m-tile geometry: tiles 0..7 are patches (128 rows), tile 8 is specials (8)
    def mslice(i):
        return slice(i * P, i * P + (P if i < NT else SPECIAL))

    def mp(i):
        return P if i < NT else SPECIAL

    with ExitStack() as ctx:
        ctx.enter_context(nc.allow_low_precision(reason="bf16 compute validated against f32 reference"))
        tc = ctx.enter_context(tile.TileContext(nc))
        persist = ctx.enter_context(tc.tile_pool(name="persist", bufs=1))
        temps = ctx.enter_context(tc.tile_pool(name="temps", bufs=3))
        ptp = ctx.enter_context(tc.tile_pool(name="ptp", bufs=10))
        oup = ctx.enter_context(tc.tile_pool(name="oup", bufs=6))
        ropep = ctx.enter_context(tc.tile_pool(name="ropep", bufs=4))
        temps2 = ctx.enter_context(tc.tile_pool(name="temps2", bufs=2))
        psum = ctx.enter_context(tc.tile_pool(name="psum", bufs=8, space="PSUM"))

        # ---- resident SBUF tensors -------------------------------------
        xT_sb = persist.tile([P, NC_, N], dt.bfloat16)
        wq_sb = persist.tile([P, NC_, 3 * DIM], dt.bfloat16)
        wo_sb = persist.tile([P, NC_, DIM], dt.bfloat16)
        for c in range(NC_):
            nc.sync.dma_start(xT_sb[:, c, :], xT[:, c, :])
            nc.sync.dma_start(wq_sb[:, c, 0:1536], wqkv[:, c, 0:1536])
            nc.sync.dma_start(wq_sb[:, c, 1536:3072], wqkv[:, c, 1536:3072])
            nc.sync.dma_start(wo_sb[:, c, :], wo[:, c, :])
        tab = {}
        for nm, ap in (("cosq", cosq), ("sinq", sinq), ("cosk", cosk), ("sink", sink)):
            t = persist.tile([P, NT + 1, HD], dt.bfloat16, tag=f"tab_{nm}")
            nc.sync.dma_start(t[:], ap[:])
            tab[nm] = t
        msk_sb = persist.tile([P, 3 * P], dt.bfloat16)
        nc.sync.dma_start(msk_sb[:], msk[:])

        qT_sb = persist.tile([P, NC_, N], dt.bfloat16, tag="qT")
        kT_sb = persist.tile([P, NC_, N], dt.bfloat16, tag="kT")
        oT_sb = persist.tile([P, NC_, N], dt.bfloat16, tag="oT")
        # V with an interleaved ones column: [128, 9 m-tiles, 16 heads, 65]
        v_sb = persist.tile([P, NT + 1, HEADS, HD + 1], dt.bfloat16, tag="v")
        nc.vector.memset(v_sb[:, :, :, HD : HD + 1], 1.0)

        ident = persist.tile([P, P], dt.bfloat16, tag="ident")
        make_identity(nc, ident[:])
        onesT = persist.tile([97, HD], dt.bfloat16, tag="onesT")
        nc.vector.memset(onesT[:], 1.0)
        eps_sb = persist.tile([P, 1], dt.float32, tag="eps")
        nc.vector.memset(eps_sb[:], EPS)

        # ---- phase A: QKV projection + RMS norm + RoPE + transpose -----
        rope_pending = []

        def flush_transposes():
            for (ii, rope, dstT) in rope_pending:
                mm = mp(ii)
                mss = mslice(ii)
                for c2 in range(NC_):
                    ptr = psum.tile([P, 512], dt.bfloat16, tag="bank", name=f"tr{ii}_{c2}")
                    nc.tensor.transpose(
                        ptr[:P, :mm], rope[:mm, c2 * P : (c2 + 1) * P], ident[:mm, :mm]
                    )
                    nc.vector.tensor_copy(dstT[:, c2, mss], ptr[:P, :mm])
            rope_pending.clear()

        for i in range(NT + 1):
            m = mp(i)
            ms = mslice(i)
            ps_j = []
            for j in range(6):
                ps_j.append(psum.tile([P, 512], dt.float32, tag="bank", name=f"qkv_ps{j}"))
            for c in range(NC_):
                lhsT = xT_sb[:, c, ms]
                for j in range(6):
                    nc.tensor.matmul(
                        ps_j[j][:m, :],
                        lhsT,
                        wq_sb[:, c, j * 512 : (j + 1) * 512],
                        start=(c == 0),
                        stop=(c == NC_ - 1),
                    )
            flush_transposes()
            # V: copy into interleaved [head, 65] layout
            for j in (4, 5):
                nc.scalar.copy(
                    v_sb[:m, i, (j - 4) * 8 : (j - 4) * 8 + 8, 0:HD],
                    ps_j[j][:m, :].rearrange("p (h d) -> p h d", h=8),
                )
            # Q (j=0,1) and K (j=2,3): norm + rope + transpose
            for which, (j0, cosn, sinn, dstT) in (
                ("q", (0, "cosq", "sinq", qT_sb)),
                ("k", (2, "cosk", "sink", kT_sb)),
            ):
                raw = temps2.tile([P, DIM], dt.bfloat16, tag="raw")
                for j in (j0, j0 + 1):
                    nc.scalar.copy(
                        raw[:m, (j - j0) * 512 : (j - j0 + 1) * 512], ps_j[j][:m, :]
                    )
                sq = temps2.tile([P, DIM], dt.bfloat16, tag="tsin")
                nc.scalar.activation(sq[:m], raw[:m], AF.Square)
                ssum = temps.tile([P, HEADS], dt.float32, tag="ssum")
                nc.vector.reduce_sum(
                    ssum[:m],
                    sq[:m].rearrange("p (h d) -> p h d", h=HEADS),
                    axis=mybir.AxisListType.X,
                )
                rstd = temps.tile([P, HEADS], dt.float32, tag="rstd")
                nc.scalar.activation(rstd[:m], ssum[:m], AF.Sqrt, bias=eps_sb[:m], scale=1.0 / HD)
                rst = temps.tile([P, HEADS], dt.bfloat16, tag="rst")
                nc.vector.reciprocal(rst[:m], rstd[:m])
                rv = raw[:m].rearrange("p (h two half) -> p h two half", h=HEADS, two=2)
                cosw = tab[cosn][:m, i, None, :].to_broadcast((m, HEADS, HD))
                sin0 = tab[sinn][:m, i, None, 0 : HD // 2].to_broadcast((m, HEADS, HD // 2))
                sin1 = tab[sinn][:m, i, None, HD // 2 : HD].to_broadcast((m, HEADS, HD // 2))
                tc_t = temps2.tile([P, DIM], dt.bfloat16, tag="tcos")
                nc.vector.tensor_tensor(
                    tc_t[:m].rearrange("p (h d) -> p h d", h=HEADS),
                    raw[:m].rearrange("p (h d) -> p h d", h=HEADS),
                    cosw,
                    op=MUL,
                )
                ts_t = temps2.tile([P, DIM], dt.bfloat16, tag="tsin")
                tsv = ts_t[:m].rearrange("p (h two half) -> p h two half", h=HEADS, two=2)
                nc.vector.tensor_tensor(tsv[:, :, 0, :], rv[:, :, 1, :], sin0, op=MUL)
                nc.vector.tensor_tensor(tsv[:, :, 1, :], rv[:, :, 0, :], sin1, op=MUL)
                nc.vector.tensor_tensor(tc_t[:m], tc_t[:m], ts_t[:m], op=ADD)
                rope = ropep.tile([P, DIM], dt.bfloat16, tag="rope")
                nc.vector.tensor_tensor(
                    rope[:m].rearrange("p (h d) -> p h d", h=HEADS),
                    tc_t[:m].rearrange("p (h d) -> p h d", h=HEADS),
                    rst[:m, :, None].to_broadcast((m, HEADS, HD)),
                    op=MUL,
                )
                rope_pending.append((i, rope, dstT))

        flush_transposes()

        # ---- phase B: banded attention, per head -----------------------
        norm_pending = [None]
        for h in range(HEADS):
            pb = 64 * (h % 2)
            ch = h // 2
            qTh = qT_sb[pb : pb + HD, ch, :]
            kTh = kT_sb[pb : pb + HD, ch, :]

            # S^T computed per k-tile s against its contiguous 384-wide q-window
            # (one matmul per s), cols (t-s+1)*128 hold q-tile t
            ps_sp = psum.tile([P, 512], dt.float32, tag="bank")  # special queries
            pts = []
            for s in range(NT):
                lo = P if s == 0 else 0
                hi = 256 if s == NT - 1 else 384
                q0 = (s - 1) * P + lo
                lhsT = kTh[:, s * P : (s + 1) * P]
                st = psum.tile([P, 512], dt.float32, tag="bank", name=f"st{s}")
                nc.tensor.matmul(
                    st[:P, lo:hi], lhsT, qTh[:, q0 : q0 + hi - lo], start=True, stop=True
                )
                nc.tensor.matmul(
                    ps_sp[:P, s * SPECIAL : (s + 1) * SPECIAL],
                    lhsT,
                    qTh[:, NP : NP + SPECIAL],
                    start=True,
                    stop=True,
                )
                ptt = ptp.tile([P, 512], dt.bfloat16, tag="pt", name=f"pt{s}")
                nc.scalar.activation(ptt[:P, lo:hi], st[:P, lo:hi], AF.Exp, scale=0.125)
                nc.gpsimd.tensor_tensor(
                    ptt[:P, lo:hi], ptt[:P, lo:hi], msk_sb[:, lo:hi], op=MUL
                )
                pts.append(ptt)
            lhsT_s = kTh[:, NP : NP + SPECIAL]
            ps_spk = [
                psum.tile([SPECIAL, 512], dt.float32, tag="bank", name=f"spk{_j}")
                for _j in range(2)
            ]
            for _j in range(2):
                nc.tensor.matmul(
                    ps_spk[_j][:SPECIAL, :], lhsT_s, qTh[:, _j * 512 : (_j + 1) * 512],
                    start=True, stop=True,
                )
            nc.tensor.matmul(
                ps_sp[:SPECIAL, NT * SPECIAL : NT * SPECIAL + SPECIAL],
                lhsT_s,
                qTh[:, NP : NP + SPECIAL],
                start=True,
                stop=True,
            )
            pt_spk = temps.tile([SPECIAL, NP], dt.bfloat16, tag="ptspk")
            for _j in range(2):
                nc.scalar.activation(
                    pt_spk[:SPECIAL, _j * 512 : (_j + 1) * 512], ps_spk[_j][:SPECIAL, :],
                    AF.Exp, scale=0.125,
                )
            pt_sp = ptp.tile([P, 512], dt.bfloat16, tag="pt")
            nc.scalar.activation(
                pt_sp[:P, 0 : NT * SPECIAL], ps_sp[:P, 0 : NT * SPECIAL], AF.Exp, scale=0.125
            )
            nc.scalar.activation(
                pt_sp[:SPECIAL, NT * SPECIAL : (NT + 1) * SPECIAL],
                ps_sp[:SPECIAL, NT * SPECIAL : (NT + 1) * SPECIAL],
                AF.Exp,
                scale=0.125,
            )

            # PV: O^T(+denominator row) = [V | 1]^T @ P^T
            den0 = temps.tile([97, P], dt.float32, tag="den0")
            den1 = temps.tile([97, P], dt.float32, tag="den1")
            nc.vector.memset(den0[:], 1.0)
            nc.vector.memset(den1[:], 1.0)
            rec0 = temps.tile([97, P], dt.bfloat16, tag="rec0")
            rec1 = temps.tile([97, P], dt.bfloat16, tag="rec1")
            po_all = []
            # bank-wide PV: one PSUM bank accumulates O^T for 4 q-tiles; windows
            # land at column offsets, per-element has_written handles first-touch
            po_b = [
                psum.tile([P, 512], dt.float32, tag="bank", name=f"pob{_b}")
                for _b in range(2)
            ]
            for b in (0, 1):
                # full-width specials matmul first: start=True covers the whole
                # bank view so later offset windows accumulate on written psum
                nc.tensor.matmul(
                    po_b[b][: HD + 1, :512],
                    v_sb[:SPECIAL, NT, h, :],
                    pt_spk[:SPECIAL, b * 512 : (b + 1) * 512],
                    start=True,
                    stop=False,
                    skip_group_check=True,
                )
            last = {0: None, 1: None}
            for s in range(NT):
                w0 = max(0, (s - 1) * P)
                w1 = min(NP, (s + 2) * P)
                for b in (0, 1):
                    if max(w0, b * 512) < min(w1, b * 512 + 512):
                        last[b] = s
            ou_b = []
            for s in range(NT):
                w0 = max(0, (s - 1) * P)
                w1 = min(NP, (s + 2) * P)
                for b in (0, 1):
                    lo = max(w0, b * 512)
                    hi = min(w1, b * 512 + 512)
                    if lo < hi:
                        nc.tensor.matmul(
                            po_b[b][: HD + 1, lo - b * 512 : hi - b * 512],
                            v_sb[:, s, h, :],
                            pts[s][:P, lo - (s - 1) * P : hi - (s - 1) * P],
                            start=False,
                            stop=(s == last[b]),
                            skip_group_check=True,
                        )
            for b in (0, 1):
                for tq in range(4):
                    t = 4 * b + tq
                    dtile = den0 if t < 4 else den1
                    nc.scalar.copy(
                        dtile[32 * tq : 32 * tq + 1, :P],
                        po_b[b][HD : HD + 1, tq * P : (tq + 1) * P],
                    )
                ou = oup.tile([HD, 512], dt.bfloat16, tag="ou", name=f"oub{b}")
                nc.scalar.copy(ou[:HD, :], po_b[b][:HD, :])
                ou_b.append(ou)
            for t in range(NT):
                po_all.append(ou_b[t // 4][:HD, (t % 4) * P : (t % 4 + 1) * P])
            # t = 8: special queries
            m = SPECIAL
            po = psum.tile([P, 512], dt.float32, tag="bank", name="po8")
            for k, s in enumerate(range(NT)):
                nc.tensor.matmul(
                    po[: HD + 1, :m], v_sb[:, s, h, :],
                    pt_sp[:P, s * SPECIAL : (s + 1) * SPECIAL],
                    start=(k == 0), stop=False,
                )
            nc.tensor.matmul(
                po[: HD + 1, :m], v_sb[:SPECIAL, NT, h, :],
                pt_sp[:SPECIAL, NT * SPECIAL : NT * SPECIAL + m],
                start=False, stop=True,
            )
            rec8 = temps.tile([1, SPECIAL], dt.bfloat16, tag="rec8")
            nc.vector.reciprocal(rec8[0:1, :m], po[HD : HD + 1, :m])
            ou8 = oup.tile([HD, P], dt.bfloat16, tag="ou8", name="ou8")
            nc.scalar.copy(ou8[:HD, :m], po[:HD, :m])
            po_all.append(ou8[:HD, :SPECIAL])
            # batched exact reciprocals: 4 q-tiles per op at bases 0/32/64/96
            nc.vector.reciprocal(rec0[:], den0[:])
            nc.vector.reciprocal(rec1[:], den1[:])

            def make_norm(pb=pb, ch=ch, po_all=po_all, rec0=rec0, rec1=rec1, rec8=rec8):
                def emit():
                    for t in range(NT + 1):
                        m = mp(t)
                        ouap = po_all[t]
                        if t < NT:
                            rtile = rec0 if t < 4 else rec1
                            base = 32 * (t % 4)
                            rrow = rtile[base : base + 1, :m]
                        else:
                            base = 0
                            rrow = rec8[0:1, :m]
                        pb2 = psum.tile([P, 512], dt.float32, tag="bank", name=f"pb2_{t}")
                        nc.tensor.matmul(
                            pb2[:HD, :m], onesT[base : base + 1, :HD], rrow,
                            start=True, stop=True, tile_position=(base, 0),
                        )
                        nc.vector.tensor_tensor(
                            oT_sb[pb : pb + HD, ch, mslice(t)], ouap, pb2[:HD, :m],
                            op=MUL,
                        )
                return emit

            if norm_pending[0] is not None:
                norm_pending[0]()
            norm_pending[0] = make_norm()

        if norm_pending[0] is not None:
            norm_pending[0]()

        # ---- phase C: out projection -----------------------------------
        for i in range(NT + 1):
            m = mp(i)
            row0 = SPECIAL + i * P if i < NT else 0
            for j in range(2):
                py = psum.tile([P, 512], dt.float32, tag="bank")
                for c in range(NC_):
                    nc.tensor.matmul(
                        py[:m, :],
                        oT_sb[:, c, mslice(i)],
                        wo_sb[:, c, j * 512 : (j + 1) * 512],
                        start=(c == 0),
                        stop=(c == NC_ - 1),
                    )
                y = temps.tile([P, 512], dt.float32, tag="y")
                nc.scalar.copy(y[:m, :], py[:m, :])
                nc.sync.dma_start(out[row0 : row0 + m, j * 512 : (j + 1) * 512], y[:m, :])

    nc.compile()
    return nc


def _get_compiled():
    global _COMPILED
    if _COMPILED is None:
        _COMPILED = _build()
    return _COMPILED


def _tile_cm(a2d, nchunks):
    """[K, F] -> [128, K//128, F] with element [p, c, f] = a2d[c*128+p, f]."""
    K, F = a2d.shape
    return np.ascontiguousarray(
        a2d.reshape(nchunks, P, F).transpose(1, 0, 2)
    )


def _prep(freqs_cos, freqs_sin, qkv_w, out_w, norm_q_w, norm_k_w):
    perm = np.concatenate([np.arange(SPECIAL, N), np.arange(0, SPECIAL)])
    wqkv_t = _tile_cm(np.asarray(qkv_w, np.float32).T.astype(bf16), NC_)
    wo_t = _tile_cm(np.asarray(out_w, np.float32).T.astype(bf16), NC_)

    c_r = np.asarray(freqs_cos, np.float32)[perm]  # [1032, 64] in m-order
    s_r = np.asarray(freqs_sin, np.float32)[perm]
    h2 = HD // 2

    def fold(w):
        w = np.asarray(w, np.float32)
        cw = c_r * w[None, :]
        sw = np.empty_like(s_r)
        sw[:, :h2] = -s_r[:, :h2] * w[None, h2:]
        sw[:, h2:] = s_r[:, h2:] * w[None, :h2]
        return cw, sw

    cq, sq_ = fold(norm_q_w)
    ck, sk_ = fold(norm_k_w)

    def padtab(t):
        tp = np.zeros(((NT + 1) * P, HD), np.float32)
        tp[:N] = t
        return _tile_cm(tp.astype(bf16), NT + 1)

    # masks: tile[j(k-part), i(q-free)] for delta = s - t in (-1, 0, +1)
    jj, ii = np.meshgrid(np.arange(P), np.arange(P), indexing="ij")
    m3 = np.zeros((P, 3 * P), np.float32)
    for d2 in (-1, 0, 1):
        ok = (np.abs(-4 * d2 + jj // GRID - ii // GRID) <= WINDOW) & (
            np.abs(jj % GRID - ii % GRID) <= WINDOW
        )
        m3[:, (d2 + 1) * P : (d2 + 2) * P] = ok
    return dict(
        wqkv=wqkv_t,
        wo=wo_t,
        cosq=padtab(cq),
        sinq=padtab(sq_),
        cosk=padtab(ck),
        sink=padtab(sk_),
        msk=m3.astype(bf16),
    )


def make_in_maps(hidden_states, freqs_cos, freqs_sin, qkv_w, out_w, norm_q_w, norm_k_w):
    shared = _prep(freqs_cos, freqs_sin, qkv_w, out_w, norm_q_w, norm_k_w)
    perm = np.concatenate([np.arange(SPECIAL, N), np.arange(0, SPECIAL)])
    hs = np.asarray(hidden_states, np.float32)
    in_maps = []
    for b in range(B):
        xb = hs[b][perm]                       # [1032, 1024] m-order
        xT = _tile_cm(np.ascontiguousarray(xb.T).astype(bf16), NC_)  # [128, 8, 1032]
        in_maps.append(dict(shared, xT=xT))
    return in_maps


def kernel(hidden_states, freqs_cos, freqs_sin, qkv_w, out_w, norm_q_w, norm_k_w):
    from concourse.bass_utils import run_bass_kernel_spmd

    nc = _get_compiled()
    in_maps = make_in_maps(
        hidden_states, freqs_cos, freqs_sin, qkv_w, out_w, norm_q_w, norm_k_w
    )
    res = run_bass_kernel_spmd(nc, in_maps, core_ids=list(range(B)))
    return np.stack([np.asarray(res.results[i]["out"], np.float32) for i in range(B)])
